# revision 1
# baseline (speedup 1.0000x reference)
"""NTN kernel for Trainium2 (8 NeuronCores, data-parallel over rows of x1).

Math: the reference collapses to
    M  = V[:, :D] + einsum('kde,e->kd', W, x2[0])          # (K, D)
    c  = x2 @ V[:, D:].T + b                               # (K,)
    y  = relu(x1 @ M.T + c) @ U                            # (N, 1)

Using relu(z + c) = max(z, -c) + c:
    y[r] = sum_k U[k] * max(z[r,k], -c[k])  +  sum_k U[k]*c[k]
The last term is a host-side scalar constant.

Device layout: x1 is transposed on host so each core receives
xt = x1_shard.T with shape [128 (=D partitions), RPC rows]. Each 128-row
tile of x1 is then directly a valid stationary (lhsT) matmul operand with
the contraction dim D on partitions; the moving operand is the tiny
Mt = M.T [128, 16] (free dim 16). PSUM accumulates 32 tiles -> [128, 32, 16],
then VectorE does max(-c) -> *U -> reduce_k, giving 32 outputs per
partition per chunk. Output [128, 489] is unshuffled on host.
"""

import numpy as np

import concourse.bass as bass
import concourse.bacc as bacc
import concourse.mybir as mybir
import concourse.tile as tile

N, D, K = 500000, 128, 16
NCORES = 8
ROWS_PER_CORE = N // NCORES          # 62500 real rows per core
TILES = 489                          # ceil(62500 / 128)
RPC = TILES * 128                    # 62592 padded rows per core
CHUNK = 32                           # 128-row tiles per chunk (one PSUM bank)
F32 = mybir.dt.float32


def _build_program():
    nc = bacc.Bacc(None, target_bir_lowering=False)

    xt = nc.dram_tensor("xt", [128, RPC], F32, kind="ExternalInput")
    mt = nc.dram_tensor("mt", [128, K], F32, kind="ExternalInput")
    negc = nc.dram_tensor("negc", [128, CHUNK, K], F32, kind="ExternalInput")
    ub = nc.dram_tensor("ub", [128, CHUNK, K], F32, kind="ExternalInput")
    y = nc.dram_tensor("y", [128, TILES], F32, kind="ExternalOutput")

    with tile.TileContext(nc) as tc:
        with (
            tc.tile_pool(name="singles", bufs=1) as singles,
            tc.tile_pool(name="xin", bufs=3) as xin,
            tc.tile_pool(name="zp", bufs=4, space="PSUM") as zpool,
            tc.tile_pool(name="work", bufs=3) as work,
            tc.tile_pool(name="yout", bufs=1) as yout,
        ):
            mt_sb = singles.tile([128, K], F32)
            nc.sync.dma_start(mt_sb, mt[:, :])
            negc_sb = singles.tile([128, CHUNK, K], F32)
            nc.sync.dma_start(negc_sb, negc[:, :, :])
            ub_sb = singles.tile([128, CHUNK, K], F32)
            nc.sync.dma_start(ub_sb, ub[:, :, :])

            y_sb = yout.tile([128, TILES], F32)

            t0 = 0
            while t0 < TILES:
                nt = min(CHUNK, TILES - t0)
                xtile = xin.tile([128, CHUNK * 128], F32, tag="xtile")
                nc.sync.dma_start(
                    xtile[:, : nt * 128], xt[:, t0 * 128 : (t0 + nt) * 128]
                )
                zp = zpool.tile([128, CHUNK, K], F32, tag="z")
                for t in range(nt):
                    nc.tensor.matmul(
                        zp[:, t, :],
                        xtile[:, t * 128 : (t + 1) * 128],
                        mt_sb[:, :],
                        start=(t == 0),
                        stop=(t == nt - 1),
                    )
                relu = work.tile([128, CHUNK, K], F32, tag="relu")
                nc.vector.tensor_tensor(
                    relu[:, :nt, :], zp[:, :nt, :], negc_sb[:, :nt, :],
                    op=mybir.AluOpType.max,
                )
                prod = work.tile([128, CHUNK, K], F32, tag="prod")
                nc.vector.tensor_tensor(
                    prod[:, :nt, :], relu[:, :nt, :], ub_sb[:, :nt, :],
                    op=mybir.AluOpType.mult,
                )
                nc.vector.tensor_reduce(
                    y_sb[:, t0 : t0 + nt], prod[:, :nt, :],
                    axis=mybir.AxisListType.X, op=mybir.AluOpType.add,
                )
                t0 += nt

            nc.sync.dma_start(y[:, :], y_sb[:, :])

    nc.compile()
    return nc


_NC_CACHE = None


def _get_program():
    global _NC_CACHE
    if _NC_CACHE is None:
        _NC_CACHE = _build_program()
    return _NC_CACHE


def _host_prep(x1, x2, V, W, b, U):
    """Fold the tiny params; shard + transpose x1. Returns in_maps + const."""
    x1 = np.asarray(x1, dtype=np.float32)
    x2 = np.asarray(x2, dtype=np.float64)
    V = np.asarray(V, dtype=np.float64)
    W = np.asarray(W, dtype=np.float64)
    b = np.asarray(b, dtype=np.float64)
    U = np.asarray(U, dtype=np.float64)

    M = V[:, :D] + np.einsum("kde,e->kd", W, x2[0])        # (K, D)
    c = (x2[0] @ V[:, D:].T) + b                           # (K,)
    u = U[:, 0]                                            # (K,)
    const = float(np.dot(u, c))

    mt = np.ascontiguousarray(M.T, dtype=np.float32)       # (128, K)
    negc_t = np.broadcast_to(
        (-c).astype(np.float32), (128, CHUNK, K)
    ).copy()
    ub_t = np.broadcast_to(u.astype(np.float32), (128, CHUNK, K)).copy()

    in_maps = []
    for cidx in range(NCORES):
        sl = x1[cidx * ROWS_PER_CORE : (cidx + 1) * ROWS_PER_CORE]
        buf = np.zeros((128, RPC), dtype=np.float32)
        buf[:, :ROWS_PER_CORE] = sl.T
        in_maps.append({"xt": buf, "mt": mt, "negc": negc_t, "ub": ub_t})
    return in_maps, const


def _gather(results, const):
    outs = []
    for cidx in range(NCORES):
        yc = np.asarray(results[cidx]["y"])                # (128, TILES)
        outs.append(yc.T.reshape(-1)[:ROWS_PER_CORE])      # row r = 128*j + p
    yfull = np.concatenate(outs) + np.float32(const)
    return yfull.reshape(N, 1).astype(np.float32)


def run_device(in_maps, trace=False):
    from concourse.bass_utils import run_bass_kernel_spmd

    nc = _get_program()
    res = run_bass_kernel_spmd(
        nc, in_maps, core_ids=list(range(NCORES)), trace=trace
    )
    return res


def kernel(x1, x2, V, W, b, U):
    in_maps, const = _host_prep(x1, x2, V, W, b, U)
    res = run_device(in_maps, trace=False)
    return _gather(res.results, const)


# revision 2
# speedup vs baseline: 2.0387x; 2.0387x over previous
"""NTN kernel for Trainium2 (8 NeuronCores, data-parallel over rows of x1).

Math: the reference collapses to
    M  = V[:, :D] + einsum('kde,e->kd', W, x2[0])          # (K, D)
    c  = x2 @ V[:, D:].T + b                               # (K,)
    y  = relu(x1 @ M.T + c) @ U                            # (N, 1)

Using relu(z + c) = max(z, -c) + c:
    y[r] = sum_k U[k] * max(z[r,k], -c[k])  +  sum_k U[k]*c[k]
The last term is a host-side scalar constant.

Device layout: x1 is transposed on host so each core receives x1_shard.T
with shape [128 (=D partitions), RPC rows]. Each 128-row tile of x1 is
then directly a valid stationary (lhsT) matmul operand with the
contraction dim D on partitions; the moving operand is the tiny
Mt = M.T [128, 16] (free dim 16).

fp32 matmuls on TRN2 run at 1/4 stream rate and pay a double-pass fp32
weight load (~427 ns/tile measured, PE-bound at 209 us). Instead x1 and
M are split hi/lo into bf16 on host (x = xh + xl exactly rounded):
    z = xh@Mh + xh@Ml + xl@Mh   (the xl@Ml term is ~2^-16 relative, dropped)
bf16 products are exact in fp32 PSUM accumulation, so the only errors are
the dropped term and fp32 accumulation rounding (~1e-5 overall).

PSUM accumulates 32 tiles -> [128, 32, 16]; VectorE does
max(-c) -> *U -> reduce_k, 32 outputs per partition per group. The
[128, 489] output is unshuffled on host (row r of this core = 128*j + p).
"""

import numpy as np
import ml_dtypes

import concourse.bass as bass
import concourse.bacc as bacc
import concourse.mybir as mybir
import concourse.tile as tile

N, D, K = 500000, 128, 16
NCORES = 8
ROWS_PER_CORE = N // NCORES          # 62500 real rows per core
TILES = 489                          # ceil(62500 / 128)
RPC = TILES * 128                    # 62592 padded rows per core
GROUP = 32                           # tiles per PSUM bank group
DMA_CHUNK = 64                       # tiles per input DMA (4 MB per chunk)
F32 = mybir.dt.float32
BF16 = mybir.dt.bfloat16
BF = ml_dtypes.bfloat16


def _build_program():
    nc = bacc.Bacc(None, target_bir_lowering=False)

    xh = nc.dram_tensor("xh", [128, RPC], BF16, kind="ExternalInput")
    xl = nc.dram_tensor("xl", [128, RPC], BF16, kind="ExternalInput")
    mth = nc.dram_tensor("mth", [128, K], BF16, kind="ExternalInput")
    mtl = nc.dram_tensor("mtl", [128, K], BF16, kind="ExternalInput")
    negc = nc.dram_tensor("negc", [128, GROUP, K], F32, kind="ExternalInput")
    ub = nc.dram_tensor("ub", [128, GROUP, K], F32, kind="ExternalInput")
    y = nc.dram_tensor("y", [128, TILES], F32, kind="ExternalOutput")

    with tile.TileContext(nc) as tc:
        with (
            tc.tile_pool(name="singles", bufs=1) as singles,
            tc.tile_pool(name="xin", bufs=3) as xin,
            tc.tile_pool(name="zp", bufs=4, space="PSUM") as zpool,
            tc.tile_pool(name="work", bufs=3) as work,
            tc.tile_pool(name="yout", bufs=1) as yout,
        ):
            mth_sb = singles.tile([128, K], BF16)
            nc.sync.dma_start(mth_sb, mth[:, :])
            mtl_sb = singles.tile([128, K], BF16)
            nc.sync.dma_start(mtl_sb, mtl[:, :])
            negc_sb = singles.tile([128, GROUP, K], F32)
            nc.sync.dma_start(negc_sb, negc[:, :, :])
            ub_sb = singles.tile([128, GROUP, K], F32)
            nc.sync.dma_start(ub_sb, ub[:, :, :])

            y_sb = yout.tile([128, TILES], F32)

            c0 = 0
            while c0 < TILES:
                nct = min(DMA_CHUNK, TILES - c0)
                xh_t = xin.tile([128, DMA_CHUNK * 128], BF16, tag="xh")
                nc.sync.dma_start(
                    xh_t[:, : nct * 128], xh[:, c0 * 128 : (c0 + nct) * 128]
                )
                xl_t = xin.tile([128, DMA_CHUNK * 128], BF16, tag="xl")
                nc.scalar.dma_start(
                    xl_t[:, : nct * 128], xl[:, c0 * 128 : (c0 + nct) * 128]
                )

                g0 = 0
                while g0 < nct:
                    nt = min(GROUP, nct - g0)
                    t0 = c0 + g0
                    zp = zpool.tile([128, GROUP, K], F32, tag="z")
                    for t in range(nt):
                        sl = slice((g0 + t) * 128, (g0 + t + 1) * 128)
                        first = t == 0
                        last = t == nt - 1
                        nc.tensor.matmul(
                            zp[:, t, :], xh_t[:, sl], mth_sb[:, :],
                            start=first, stop=False,
                        )
                        nc.tensor.matmul(
                            zp[:, t, :], xh_t[:, sl], mtl_sb[:, :],
                            start=False, stop=False,
                        )
                        nc.tensor.matmul(
                            zp[:, t, :], xl_t[:, sl], mth_sb[:, :],
                            start=False, stop=last,
                        )
                    relu = work.tile([128, GROUP, K], F32, tag="relu")
                    nc.vector.tensor_tensor(
                        relu[:, :nt, :], zp[:, :nt, :], negc_sb[:, :nt, :],
                        op=mybir.AluOpType.max,
                    )
                    prod = work.tile([128, GROUP, K], F32, tag="prod")
                    nc.vector.tensor_tensor(
                        prod[:, :nt, :], relu[:, :nt, :], ub_sb[:, :nt, :],
                        op=mybir.AluOpType.mult,
                    )
                    nc.vector.tensor_reduce(
                        y_sb[:, t0 : t0 + nt], prod[:, :nt, :],
                        axis=mybir.AxisListType.X, op=mybir.AluOpType.add,
                    )
                    g0 += nt
                c0 += nct

            nc.sync.dma_start(y[:, :], y_sb[:, :])

    nc.compile()
    return nc


_NC_CACHE = None


def _get_program():
    global _NC_CACHE
    if _NC_CACHE is None:
        _NC_CACHE = _build_program()
    return _NC_CACHE


def _host_prep(x1, x2, V, W, b, U):
    """Fold the tiny params; shard + hi/lo-split + transpose x1."""
    x1 = np.asarray(x1, dtype=np.float32)
    x2 = np.asarray(x2, dtype=np.float64)
    V = np.asarray(V, dtype=np.float64)
    W = np.asarray(W, dtype=np.float64)
    b = np.asarray(b, dtype=np.float64)
    U = np.asarray(U, dtype=np.float64)

    M = V[:, :D] + np.einsum("kde,e->kd", W, x2[0])        # (K, D)
    c = (x2[0] @ V[:, D:].T) + b                           # (K,)
    u = U[:, 0]                                            # (K,)
    const = float(np.dot(u, c))

    Mh = M.astype(BF)
    Ml = (M - Mh.astype(np.float64)).astype(BF)
    mth = np.ascontiguousarray(Mh.T)                       # (128, K) bf16
    mtl = np.ascontiguousarray(Ml.T)
    negc_t = np.broadcast_to(
        (-c).astype(np.float32), (128, GROUP, K)
    ).copy()
    ub_t = np.broadcast_to(u.astype(np.float32), (128, GROUP, K)).copy()

    in_maps = []
    for cidx in range(NCORES):
        sl = x1[cidx * ROWS_PER_CORE : (cidx + 1) * ROWS_PER_CORE]
        slt = sl.T                                         # (128, 62500) view
        hi = slt.astype(BF)
        lo = (slt - hi.astype(np.float32)).astype(BF)
        hbuf = np.zeros((128, RPC), dtype=BF)
        hbuf[:, :ROWS_PER_CORE] = hi
        lbuf = np.zeros((128, RPC), dtype=BF)
        lbuf[:, :ROWS_PER_CORE] = lo
        in_maps.append(
            {"xh": hbuf, "xl": lbuf, "mth": mth, "mtl": mtl,
             "negc": negc_t, "ub": ub_t}
        )
    return in_maps, const


def _gather(results, const):
    outs = []
    for cidx in range(NCORES):
        yc = np.asarray(results[cidx]["y"])                # (128, TILES)
        outs.append(yc.T.reshape(-1)[:ROWS_PER_CORE])      # row r = 128*j + p
    yfull = np.concatenate(outs) + np.float32(const)
    return yfull.reshape(N, 1).astype(np.float32)


def run_device(in_maps, trace=False):
    from concourse.bass_utils import run_bass_kernel_spmd

    nc = _get_program()
    res = run_bass_kernel_spmd(
        nc, in_maps, core_ids=list(range(NCORES)), trace=trace
    )
    return res


def kernel(x1, x2, V, W, b, U):
    in_maps, const = _host_prep(x1, x2, V, W, b, U)
    res = run_device(in_maps, trace=False)
    return _gather(res.results, const)


# revision 4
# speedup vs baseline: 2.0415x; 1.0014x over previous
"""NTN kernel for Trainium2 (8 NeuronCores, data-parallel over rows of x1).

Math: the reference collapses to
    M  = V[:, :D] + einsum('kde,e->kd', W, x2[0])          # (K, D)
    c  = x2 @ V[:, D:].T + b                               # (K,)
    y  = relu(x1 @ M.T + c) @ U                            # (N, 1)

Using relu(z + c) = max(z, -c) + c:
    y[r] = sum_k U[k] * max(z[r,k], -c[k])  +  sum_k U[k]*c[k]
The last term is a host-side scalar constant.

Device layout: x1 is transposed on host so each core receives x1_shard.T
with shape [128 (=D partitions), RPC rows]. Each 128-row tile of x1 is
then directly a valid stationary (lhsT) matmul operand with the
contraction dim D on partitions; the moving operand is the tiny
Mt = M.T [128, 16] (free dim 16).

fp32 matmuls on TRN2 run at 1/4 stream rate and pay a double-pass fp32
weight load (~427 ns/tile measured, PE-bound at 209 us). Instead x1 and
M are split hi/lo into bf16 on host (x = xh + xl exactly rounded):
    z = xh@Mh + xh@Ml + xl@Mh   (the xl@Ml term is ~2^-16 relative, dropped)
bf16 products are exact in fp32 PSUM accumulation, so the only errors are
the dropped term and fp32 accumulation rounding (~1e-5 overall).

PSUM accumulates 32 tiles -> [128, 32, 16]; VectorE does
max(-c) -> *U -> reduce_k, 32 outputs per partition per group. The
[128, 489] output is unshuffled on host (row r of this core = 128*j + p).
"""

import numpy as np
import ml_dtypes

import concourse.bass as bass
import concourse.bacc as bacc
import concourse.mybir as mybir
import concourse.tile as tile

N, D, K = 500000, 128, 16
NCORES = 8
ROWS_PER_CORE = N // NCORES          # 62500 real rows per core
TILES = 489                          # ceil(62500 / 128)
RPC = TILES * 128                    # 62592 padded rows per core
GROUP = 32                           # tiles per PSUM bank group
DMA_CHUNK = 64                       # tiles per input DMA (4 MB per chunk)
F32 = mybir.dt.float32
BF16 = mybir.dt.bfloat16
BF = ml_dtypes.bfloat16


def _build_program():
    nc = bacc.Bacc(None, target_bir_lowering=False)

    xh = nc.dram_tensor("xh", [128, RPC], BF16, kind="ExternalInput")
    xl = nc.dram_tensor("xl", [128, RPC], BF16, kind="ExternalInput")
    mth = nc.dram_tensor("mth", [128, K], BF16, kind="ExternalInput")
    mtl = nc.dram_tensor("mtl", [128, K], BF16, kind="ExternalInput")
    negc = nc.dram_tensor("negc", [128, GROUP, K], F32, kind="ExternalInput")
    ub = nc.dram_tensor("ub", [128, GROUP, K], F32, kind="ExternalInput")
    y = nc.dram_tensor("y", [128, TILES], F32, kind="ExternalOutput")

    with tile.TileContext(nc) as tc:
        with (
            tc.tile_pool(name="singles", bufs=1) as singles,
            tc.tile_pool(name="xin", bufs=3) as xin,
            tc.tile_pool(name="zp", bufs=4, space="PSUM") as zpool,
            tc.tile_pool(name="work", bufs=3) as work,
            tc.tile_pool(name="yout", bufs=1) as yout,
        ):
            # Pre-issue ALL input DMAs in chunk order, params on the gpsimd
            # (SWDGE) queue so the two HWDGE queues carry nothing but the
            # bulk input stream, starting at t~0. Alternate xh/xl between
            # the two HWDGE queues per chunk so both queues carry equal
            # bytes and drain together.
            chunk_tiles = []
            c0 = 0
            while c0 < TILES:
                nct = min(DMA_CHUNK, TILES - c0)
                xh_t = xin.tile([128, DMA_CHUNK * 128], BF16, tag="xh")
                xl_t = xin.tile([128, DMA_CHUNK * 128], BF16, tag="xl")
                eng_a, eng_b = (
                    (nc.sync, nc.scalar)
                    if (len(chunk_tiles) % 2 == 0)
                    else (nc.scalar, nc.sync)
                )
                eng_a.dma_start(
                    xh_t[:, : nct * 128], xh[:, c0 * 128 : (c0 + nct) * 128]
                )
                eng_b.dma_start(
                    xl_t[:, : nct * 128], xl[:, c0 * 128 : (c0 + nct) * 128]
                )
                chunk_tiles.append((c0, nct, xh_t, xl_t))
                c0 += nct

            mth_sb = singles.tile([128, K], BF16)
            nc.gpsimd.dma_start(mth_sb, mth[:, :])
            mtl_sb = singles.tile([128, K], BF16)
            nc.gpsimd.dma_start(mtl_sb, mtl[:, :])
            negc_sb = singles.tile([128, GROUP, K], F32)
            nc.gpsimd.dma_start(negc_sb, negc[:, :, :])
            ub_sb = singles.tile([128, GROUP, K], F32)
            nc.gpsimd.dma_start(ub_sb, ub[:, :, :])

            y_sb = yout.tile([128, TILES], F32)

            for c0, nct, xh_t, xl_t in chunk_tiles:
                g0 = 0
                while g0 < nct:
                    nt = min(GROUP, nct - g0)
                    t0 = c0 + g0
                    zp = zpool.tile([128, GROUP, K], F32, tag="z")
                    for t in range(nt):
                        sl = slice((g0 + t) * 128, (g0 + t + 1) * 128)
                        first = t == 0
                        last = t == nt - 1
                        nc.tensor.matmul(
                            zp[:, t, :], xh_t[:, sl], mth_sb[:, :],
                            start=first, stop=False,
                        )
                        nc.tensor.matmul(
                            zp[:, t, :], xh_t[:, sl], mtl_sb[:, :],
                            start=False, stop=False,
                        )
                        nc.tensor.matmul(
                            zp[:, t, :], xl_t[:, sl], mth_sb[:, :],
                            start=False, stop=last,
                        )
                    relu = work.tile([128, GROUP, K], F32, tag="relu")
                    nc.vector.tensor_tensor(
                        relu[:, :nt, :], zp[:, :nt, :], negc_sb[:, :nt, :],
                        op=mybir.AluOpType.max,
                    )
                    prod = work.tile([128, GROUP, K], F32, tag="prod")
                    nc.vector.tensor_tensor(
                        prod[:, :nt, :], relu[:, :nt, :], ub_sb[:, :nt, :],
                        op=mybir.AluOpType.mult,
                    )
                    nc.vector.tensor_reduce(
                        y_sb[:, t0 : t0 + nt], prod[:, :nt, :],
                        axis=mybir.AxisListType.X, op=mybir.AluOpType.add,
                    )
                    g0 += nt

            nc.sync.dma_start(y[:, :], y_sb[:, :])

    nc.compile()
    return nc


_NC_CACHE = None


def _get_program():
    global _NC_CACHE
    if _NC_CACHE is None:
        _NC_CACHE = _build_program()
    return _NC_CACHE


def _host_prep(x1, x2, V, W, b, U):
    """Fold the tiny params; shard + hi/lo-split + transpose x1."""
    x1 = np.asarray(x1, dtype=np.float32)
    x2 = np.asarray(x2, dtype=np.float64)
    V = np.asarray(V, dtype=np.float64)
    W = np.asarray(W, dtype=np.float64)
    b = np.asarray(b, dtype=np.float64)
    U = np.asarray(U, dtype=np.float64)

    M = V[:, :D] + np.einsum("kde,e->kd", W, x2[0])        # (K, D)
    c = (x2[0] @ V[:, D:].T) + b                           # (K,)
    u = U[:, 0]                                            # (K,)
    const = float(np.dot(u, c))

    Mh = M.astype(BF)
    Ml = (M - Mh.astype(np.float64)).astype(BF)
    mth = np.ascontiguousarray(Mh.T)                       # (128, K) bf16
    mtl = np.ascontiguousarray(Ml.T)
    negc_t = np.broadcast_to(
        (-c).astype(np.float32), (128, GROUP, K)
    ).copy()
    ub_t = np.broadcast_to(u.astype(np.float32), (128, GROUP, K)).copy()

    in_maps = []
    for cidx in range(NCORES):
        sl = x1[cidx * ROWS_PER_CORE : (cidx + 1) * ROWS_PER_CORE]
        slt = sl.T                                         # (128, 62500) view
        hi = slt.astype(BF)
        lo = (slt - hi.astype(np.float32)).astype(BF)
        hbuf = np.zeros((128, RPC), dtype=BF)
        hbuf[:, :ROWS_PER_CORE] = hi
        lbuf = np.zeros((128, RPC), dtype=BF)
        lbuf[:, :ROWS_PER_CORE] = lo
        in_maps.append(
            {"xh": hbuf, "xl": lbuf, "mth": mth, "mtl": mtl,
             "negc": negc_t, "ub": ub_t}
        )
    return in_maps, const


def _gather(results, const):
    outs = []
    for cidx in range(NCORES):
        yc = np.asarray(results[cidx]["y"])                # (128, TILES)
        outs.append(yc.T.reshape(-1)[:ROWS_PER_CORE])      # row r = 128*j + p
    yfull = np.concatenate(outs) + np.float32(const)
    return yfull.reshape(N, 1).astype(np.float32)


def run_device(in_maps, trace=False):
    from concourse.bass_utils import run_bass_kernel_spmd

    nc = _get_program()
    res = run_bass_kernel_spmd(
        nc, in_maps, core_ids=list(range(NCORES)), trace=trace
    )
    return res


def kernel(x1, x2, V, W, b, U):
    in_maps, const = _host_prep(x1, x2, V, W, b, U)
    res = run_device(in_maps, trace=False)
    return _gather(res.results, const)
